# revision 7
# baseline (speedup 1.0000x reference)
"""Trainium2 Bass kernel for InterpretableMultiHeadAttention (redesign v3).

Sharding: 8 cores, head-parallel (core c owns heads {2c, 2c+1} = 128 proj
features); an AllToAll redistributes attention output row-major so core c
ends with rows 256c:256(c+1) of each batch for the output projection.

Key structure (vs 373us baseline):
  - fp8 q/k activations+weights, bf16 v/wo streaming (cuts DMA startup);
    all matmuls bf16/fp8 (full PE rate in the cost model).
  - V projected directly in [keys, features] orientation; no PE transposes.
  - RoPE: Act evacuates proj psum (+bias, ->bf16), a PE permutation matmul
    produces the rotate-half copy, DVE does cos-mul, sin-mul, combine add.
    Sin sign folded host-side.
  - Scores: bf16 matmuls into a 2-bank [128,(2,512)] psum per 256-key
    chunk-pair; ONE exp instr per pair, split between Act (AF.Exp,
    scale=1/8) and DVE (pow(e^{1/8}, x)) to break the Act exp bottleneck.
  - AV is computed TRANSPOSED: out[q-chunk 128, 66] = sum over key chunks
    of pt[keys, q-chunk].T @ v1[keys, 64 vals + ones col + zero pad].
    Free size 66 per matmul -> half the PE cycles of the feature-major
    form; denominators land in psum column 64.
  - Normalization: one per-partition tensor_scalar divide by the psum
    denominator column -> bf16 dvq [q, 64]; the row-major a2a_in write is
    a plain small DMA.
  - Receiver side: 8 XBAR DMA-transposes rebuild the feature-major at
    tile (~220ns each on the DMA engines) for the output projection.
  - Emission micro-interleaved: batch-1 projections fill PE gaps during
    batch-0 attention; next unit's scores interleave with previous unit's
    AV block; batch-0 output projection hides under the batch-1 tail.
"""

import os
import sys

import numpy as np

sys.path.insert(0, "/opt/trn_rl_repo")

import concourse.bass as bass  # noqa: E402
from concourse import bacc  # noqa: E402
import concourse.tile as tile  # noqa: E402
from concourse import mybir  # noqa: E402

F32 = mybir.dt.float32
BF16 = mybir.dt.bfloat16
F8 = mybir.dt.float8e4
AF = mybir.ActivationFunctionType
ALU = mybir.AluOpType

B = 2
D_MODEL = 1024
NHEAD = 16
HEAD_DIM = 64
N_CORES = 8
P = 128
ROPE_BASE = 10000.0

# chunk-pair indices (of 8) whose exp runs on DVE instead of Act
DVE_C = ()

TRACE = bool(int(os.environ.get("BASS_KERNEL_TRACE", "0")))
LAST_RESULTS = None


def build_nc(S=2048, T_TILE=512):
    D = D_MODEL
    KC = D // P            # proj contraction chunks (8)
    SC = S // P            # 128-key chunks (16)
    NPAIR = SC // 2        # 256-key chunk pairs (8)
    NT = S // T_TILE       # query tiles (4)
    NQ = T_TILE // P       # 128-query chunks per t-tile (4)
    TS = S // N_CORES      # rows per core after AllToAll (256)
    VH = 66                # per-head v1 stride: 64 vals + ones + zero pad

    nc = bacc.Bacc()

    xq = nc.declare_dram_parameter("xq", [B, D, S], F8, isOutput=False)
    xk = nc.declare_dram_parameter("xk", [B, D, S], F8, isOutput=False)
    xv = nc.declare_dram_parameter("xv", [B, D, S], BF16, isOutput=False)
    wq = nc.declare_dram_parameter("wq", [D, P], F8, isOutput=False)
    wk = nc.declare_dram_parameter("wk", [D, P], F8, isOutput=False)
    wv = nc.declare_dram_parameter("wv", [D, P], BF16, isOutput=False)
    bqp = nc.declare_dram_parameter("bq", [P, 1], F32, isOutput=False)
    bkp = nc.declare_dram_parameter("bk", [P, 1], F32, isOutput=False)
    bvp = nc.declare_dram_parameter("bv", [1, P], BF16, isOutput=False)
    wo = nc.declare_dram_parameter("wo", [D, D], BF16, isOutput=False)
    bop = nc.declare_dram_parameter("bo", [1, D], BF16, isOutput=False)
    cosr = nc.declare_dram_parameter("cosr", [P, S], BF16, isOutput=False)
    sinr = nc.declare_dram_parameter("sinr", [P, S], BF16, isOutput=False)
    permp = nc.declare_dram_parameter("perm", [P, P], BF16, isOutput=False)
    y = nc.declare_dram_parameter("y", [B, TS, D], F32, isOutput=True)

    with tile.TileContext(nc) as tc:
        with (
            nc.allow_low_precision(reason="bf16/fp8 attention pipeline"),
            tc.tile_pool(name="singles", bufs=1) as singles,
            tc.tile_pool(name="big", bufs=2) as big,
            tc.tile_pool(name="xt", bufs=18) as xtp,
            tc.tile_pool(name="sb", bufs=2) as sb,
            tc.tile_pool(name="psc", bufs=2, space="PSUM") as pscp,
            tc.tile_pool(name="pav", bufs=2, space="PSUM") as pavp,
            tc.tile_pool(name="pj", bufs=2, space="PSUM") as pjp,
            tc.tile_pool(name="dram", bufs=2, space="DRAM") as dram,
        ):
            # ---------------- persistent loads ----------------
            wq_sb = singles.tile([P, KC, P], F8, tag="wq", name="wq_sb")
            wk_sb = singles.tile([P, KC, P], F8, tag="wk", name="wk_sb")
            wv_sb = singles.tile([P, KC, P], BF16, tag="wv", name="wv_sb")
            nc.sync.dma_start(wq_sb, wq.rearrange("(ko p) m -> p ko m", p=P))
            nc.sync.dma_start(wk_sb, wk.rearrange("(ko p) m -> p ko m", p=P))
            perm_sb = singles.tile([P, P], BF16, tag="perm", name="perm_sb")
            nc.sync.dma_start(perm_sb, permp[:, :])
            bq_sb = singles.tile([P, 1], F32, tag="bq", name="bq_sb")
            bk_sb = singles.tile([P, 1], F32, tag="bk", name="bk_sb")
            nc.sync.dma_start(bq_sb, bqp[:, :])
            nc.sync.dma_start(bk_sb, bkp[:, :])
            bv_sb = singles.tile([1, P], BF16, tag="bv", name="bv_sb")
            cos_sb = singles.tile([P, S], BF16, tag="cos", name="cos_sb")
            sin_sb = singles.tile([P, S], BF16, tag="sin", name="sin_sb")

            def load_rest():
                # deferred off the critical k-load path: cos/sin first
                # needed at the first rope (~10us), wv at the first v chunk
                nc.sync.dma_start(cos_sb, cosr[:, :])
                nc.sync.dma_start(sin_sb, sinr[:, :])
                nc.sync.dma_start(
                    wv_sb, wv.rearrange("(ko p) m -> p ko m", p=P)
                )
                nc.sync.dma_start(bv_sb, bvp[:, :])
            ones1 = singles.tile([1, P], BF16, tag="ones1", name="ones1")
            nc.vector.memset(ones1, 1.0)
            ebase = singles.tile([P, 1], F32, tag="ebase", name="ebase")
            nc.vector.memset(ebase, float(np.exp(0.125)))
            # deferred loads (first needed at outproj time)
            wo_sb = singles.tile([P, KC, D], BF16, tag="wo", name="wo_sb")
            bo_sb = singles.tile([1, D], BF16, tag="bo", name="bo_sb")

            def load_wo():
                nc.sync.dma_start(wo_sb, wo.rearrange("(ko p) m -> p ko m", p=P))
                nc.sync.dma_start(bo_sb, bop[:, :])

            # ---------------- per-batch builders ----------------
            def make_batch_state(b):
                st = {}
                st["qT"] = big.tile([P, S], BF16, tag="qT", name=f"qT{b}")
                st["kT"] = big.tile([P, S], BF16, tag="kT", name=f"kT{b}")
                # [P=keys, key-chunk, head, VH]; col 64 = ones, 65 = zero
                st["v1"] = big.tile(
                    [P, SC, 2, VH], BF16, tag="v1", name=f"v1_{b}"
                )
                # row-major exchange: block j -> core j gets rows [256j:...)
                st["a2a_in"] = dram.tile(
                    [N_CORES, TS, P], BF16, tag="a2a_in", name=f"a2a_in{b}"
                )
                st["a2a_out"] = dram.tile(
                    [N_CORES, TS, P], BF16, tag="a2a_out", name=f"a2a_out{b}"
                )
                st["stage"] = st["a2a_in"]
                return st

            def gen_vinit(st):
                def cl():
                    nc.vector.memset(st["v1"][:, :, :, 64:65], 1.0)
                    nc.vector.memset(st["v1"][:, :, :, 65:66], 0.0)

                return [cl]

            def gen_qk_proj(b, xc, wsb, bias_sb, dst, g):
                """One 1024-col group of a q/k projection (fp8 inputs).

                g=0 loads full-width fp8 x tiles (all S cols); they stay
                live for g=1 via the per-batch cache list in xc."""
                ops = []
                pss = [None, None]
                xts = xc["tiles"]

                def load(k):
                    def cl():
                        xts[k] = xtp.tile(
                            [P, S], F8, tag="xf8", name="xf8", bufs=10
                        )
                        nc.sync.dma_start(
                            xts[k], xc["p"][b, k * P : (k + 1) * P, :]
                        )

                    return cl

                def mm(k):
                    def cl():
                        if k == 0:
                            for j in range(2):
                                pss[j] = pjp.tile(
                                    [P, T_TILE], F32, tag="pj", name="pj"
                                )
                        for j in range(2):
                            n = g * 2 + j
                            nc.tensor.matmul(
                                pss[j],
                                lhsT=wsb[:, k, :],
                                rhs=xts[k][:, n * T_TILE : (n + 1) * T_TILE],
                                start=(k == 0),
                                stop=(k == KC - 1),
                            )

                    return cl

                for k in range(KC):
                    if g == 0:
                        ops.append(load(k))
                    ops.append(mm(k))

                def rope(j):
                    def cl():
                        n = g * 2 + j
                        seg = slice(n * T_TILE, (n + 1) * T_TILE)
                        ps = pss[j]
                        qb = sb.tile([P, T_TILE], BF16, tag="qb", name="qb")
                        nc.vector.tensor_scalar(
                            out=qb, in0=ps, scalar1=bias_sb,
                            scalar2=None, op0=ALU.add,
                        )
                        psr = pjp.tile([P, T_TILE], F32, tag="pj", name="psr")
                        nc.tensor.matmul(
                            psr, lhsT=perm_sb, rhs=qb, start=True, stop=True
                        )
                        qc = sb.tile([P, T_TILE], BF16, tag="qc", name="qc")
                        nc.vector.tensor_tensor(
                            out=qc, in0=qb, in1=cos_sb[:, seg], op=ALU.mult
                        )
                        qs = sb.tile([P, T_TILE], BF16, tag="qs", name="qs")
                        nc.vector.tensor_tensor(
                            out=qs, in0=psr, in1=sin_sb[:, seg], op=ALU.mult
                        )
                        nc.vector.tensor_tensor(
                            out=dst[:, seg], in0=qc, in1=qs, op=ALU.add
                        )

                    return cl

                ops.append(rope(0))
                ops.append(rope(1))
                return ops

            def gen_v_proj(b, st, g):
                """One 1024-key group of the v projection (8 key chunks).

                xv loads go through the (otherwise idle) Pool SWDGE queue so
                their slot-waits never block SP/Act dispatch."""
                ops = []
                xts = [None] * KC

                def load(k):
                    def cl():
                        xts[k] = xtp.tile(
                            [P, 2 * T_TILE], BF16, tag="xt", name="xtv"
                        )
                        eng = nc.gpsimd if k % 2 == 0 else nc.scalar
                        eng.dma_start(
                            xts[k],
                            xv[
                                b,
                                k * P : (k + 1) * P,
                                g * 2 * T_TILE : (g + 1) * 2 * T_TILE,
                            ],
                        )

                    return cl

                for k in range(KC):
                    ops.append(load(k))

                def chunk(kc):
                    def cl():
                        s = g * KC + kc
                        psv = pjp.tile([P, P], F32, tag="pj", name="psv")
                        for k in range(KC):
                            nc.tensor.matmul(
                                psv,
                                lhsT=xts[k][:, kc * P : (kc + 1) * P],
                                rhs=wv_sb[:, k, :],
                                start=(k == 0),
                                stop=False,
                            )
                        nc.tensor.matmul(
                            psv, lhsT=ones1, rhs=bv_sb, start=False, stop=True
                        )
                        v1 = st["v1"]
                        for h in range(2):
                            nc.vector.tensor_copy(
                                v1[:, s, h, 0:64],
                                psv[:, h * 64 : h * 64 + 64],
                            )

                    return cl

                for kc in range(KC):
                    ops.append(chunk(kc))
                return ops

            def gen_attn_unit(b, st, t, h):
                """One (t-tile, head) attention unit.

                Returns (pair_ops, av_ops): pair_ops computes scores+exp for
                the 8 chunk-pairs; av_ops runs the transposed AV matmuls per
                128-query chunk, normalizes, and writes a2a_in."""
                tseg = slice(t * T_TILE, (t + 1) * T_TILE)
                r0 = h * 64
                pts = [None] * NPAIR

                pair_ops = []

                def pair(c):
                    def cl():
                        qT, kT = st["qT"], st["kT"]
                        psc = pscp.tile(
                            [P, 2, T_TILE], F32, tag="psc", name="psc"
                        )
                        for i in range(2):
                            sblk = (2 * c + i) * P
                            nc.tensor.matmul(
                                psc[:, i, :],
                                lhsT=kT[r0 : r0 + 64, sblk : sblk + P],
                                rhs=qT[r0 : r0 + 64, tseg],
                                start=True,
                                stop=True,
                            )
                        pts[c] = sb.tile(
                            [P, 2, T_TILE], BF16, tag="pt", name="pt", bufs=28
                        )
                        if c in DVE_C:
                            nc.vector.tensor_tensor(
                                out=pts[c][:, :, :],
                                in0=ebase.to_broadcast((P, 2, T_TILE)),
                                in1=psc[:, :, :],
                                op=ALU.pow,
                            )
                        else:
                            nc.scalar.activation(
                                pts[c][:, :, :], psc[:, :, :], AF.Exp,
                                scale=0.125,
                            )

                    return cl

                for c in range(NPAIR):
                    pair_ops.append(pair(c))

                av_ops = []
                pq_box = [None]

                def avq(qc):
                    def cl():
                        v1 = st["v1"]
                        if qc == 0:
                            pq_box[0] = pavp.tile(
                                [P, NQ, P], F32, tag="pav", name="pav"
                            )
                        pq = pq_box[0]
                        for s in range(SC):
                            nc.tensor.matmul(
                                pq[:, qc, 0:VH],
                                lhsT=pts[s // 2][
                                    :, s % 2, qc * P : (qc + 1) * P
                                ],
                                rhs=v1[:, s, h, :],
                                start=(s == 0),
                                stop=(s == SC - 1),
                            )
                        rcq = sb.tile([P, 1], F32, tag="rcq", name="rcq", bufs=8)
                        nc.vector.reciprocal(rcq, pq[:, qc, 64:65])
                        dvq = sb.tile(
                            [P, 64], BF16, tag="dvq", name="dvq", bufs=64
                        )
                        nc.vector.tensor_scalar(
                            out=dvq,
                            in0=pq[:, qc, 0:64],
                            scalar1=rcq,
                            scalar2=None,
                            op0=ALU.mult,
                        )
                        row = t * T_TILE + qc * P
                        j, rr = row // TS, row % TS
                        nc.sync.dma_start(
                            st["stage"][j, rr : rr + P, r0 : r0 + 64], dvq
                        )

                    return cl

                for qc in range(NQ):
                    av_ops.append(avq(qc))
                return pair_ops, av_ops

            def gen_a2a(st):
                def cl():
                    nc.gpsimd.collective_compute(
                        "AllToAll",
                        ALU.bypass,
                        replica_groups=[list(range(N_CORES))],
                        ins=[st["a2a_in"].opt()],
                        outs=[st["a2a_out"].opt()],
                    )

                return [cl]

            def gen_at_loads(b, st):
                """The 8 XBAR transposes rebuilding the feature-major at
                tile. Issue these IMMEDIATELY after the batch's a2a emission:
                DMAHW waits are completion-count based, so late issue makes
                every at consumer also wait for all intervening DMAs."""
                ops = []
                at_box = [None]

                def load_at(i):
                    def cl():
                        if i == 0:
                            at_box[0] = sb.tile(
                                [P, KC, TS], BF16, tag="at", name="at"
                            )
                        nc.sync.dma_start(
                            at_box[0][:, i, :],
                            st["a2a_out"][i, :, :],
                            transpose=True,
                        )

                    return cl

                for i in range(KC):
                    ops.append(load_at(i))
                return ops, at_box

            def gen_outproj(b, st, at_box):
                ops = []

                def block(n, m):
                    def cl():
                        at = at_box[0]
                        nseg = slice(n * T_TILE, (n + 1) * T_TILE)
                        pool = pjp if (n + m) % 2 == 0 else pscp
                        tag = "pj" if (n + m) % 2 == 0 else "psc"
                        py = pool.tile([P, T_TILE], F32, tag=tag, name="py")
                        for k in range(KC):
                            nc.tensor.matmul(
                                py,
                                lhsT=at[:, k, m * P : (m + 1) * P],
                                rhs=wo_sb[:, k, nseg],
                                start=(k == 0),
                                stop=False,
                            )
                        nc.tensor.matmul(
                            py,
                            lhsT=ones1,
                            rhs=bo_sb[:, nseg],
                            start=False,
                            stop=True,
                        )
                        ysb = sb.tile([P, T_TILE], F32, tag="ysb", name="ysb", bufs=8)
                        nc.vector.tensor_copy(ysb, py)
                        nc.sync.dma_start(y[b, m * P : (m + 1) * P, nseg], ysb)

                    return cl

                for n in range(D // T_TILE):
                    for m in range(TS // P):
                        ops.append(block(n, m))
                return ops

            def interleave(primary, secondary):
                np_, ns = len(primary), len(secondary)
                j = 0
                for i, cl in enumerate(primary):
                    cl()
                    jt = (i + 1) * ns // np_
                    while j < jt:
                        secondary[j]()
                        j += 1
                while j < ns:
                    secondary[j]()
                    j += 1

            # ---------------- schedule ----------------
            st0 = make_batch_state(0)
            st1 = make_batch_state(1)

            def proj_ops(b, st):
                """Returns (pre, rest): `pre` must finish before the batch's
                first attention pair (k fully, q tiles 0-1, v loads started);
                `rest` (q g1, v matmul chunks) streams into the attention
                interleave."""
                xkc = {"p": xk, "tiles": [None] * KC}
                xqc = {"p": xq, "tiles": [None] * KC}
                vg0 = gen_v_proj(b, st, 0)
                vg1 = gen_v_proj(b, st, 1)
                v_loads0, v_chunks0 = vg0[:KC], vg0[KC:]
                v_loads1, v_chunks1 = vg1[:KC], vg1[KC:]
                pre = []
                pre += gen_vinit(st)
                pre += v_loads0
                pre += v_loads1
                kg0 = gen_qk_proj(b, xkc, wk_sb, bk_sb, st["kT"], 0)
                if b == 0:
                    # cos/sin must land before the first rope closure
                    kg0 = kg0[:-2] + [load_rest] + kg0[-2:]
                pre += kg0
                pre += gen_qk_proj(b, xkc, wk_sb, bk_sb, st["kT"], 1)
                pre += gen_qk_proj(b, xqc, wq_sb, bq_sb, st["qT"], 0)
                rest = []
                rest += v_chunks0
                rest += v_chunks1
                rest += gen_qk_proj(b, xqc, wq_sb, bq_sb, st["qT"], 1)
                return pre, rest

            def attn_ops(b, st, av_lag=3):
                """Pair ops of unit u interleave with av ops of unit
                u-av_lag (gives v1 time to finish during early exps)."""
                ops = []
                pending = []
                for t in range(NT):
                    for h in range(2):
                        pair_ops, av_ops = gen_attn_unit(b, st, t, h)
                        if av_lag == 0:
                            ops += pair_ops
                            ops += av_ops
                            continue
                        prev_av = (
                            pending.pop(0) if len(pending) >= av_lag else []
                        )
                        j = 0
                        for i, cl in enumerate(pair_ops):
                            ops.append(cl)
                            jt = (i + 1) * len(prev_av) // len(pair_ops)
                            while j < jt:
                                ops.append(prev_av[j])
                                j += 1
                        ops.extend(prev_av[j:])
                        if av_lag > 0:
                            pending.append(av_ops)
                for av in pending:
                    ops += av
                return ops

            pre0, rest0 = proj_ops(0, st0)
            for cl in pre0:
                cl()
            load_wo()
            pre1, rest1 = proj_ops(1, st1)
            interleave(attn_ops(0, st0), rest0 + pre1 + rest1)
            interleave(attn_ops(1, st1, av_lag=1), gen_a2a(st0))
            at0_ops, at0_box = gen_at_loads(0, st0)
            for cl in at0_ops:
                cl()
            for cl in gen_outproj(0, st0, at0_box):
                cl()
            for cl in gen_a2a(st1):
                cl()
            at1_ops, at1_box = gen_at_loads(1, st1)
            for cl in at1_ops:
                cl()
            for cl in gen_outproj(1, st1, at1_box):
                cl()

    nc.compile()
    return nc


def host_inputs(query, key_, value, Wq, bq, Wk, bk, Wv, bv, Wo, bo, S=2048):
    """Per-core input maps (host-side layout/dtype prep)."""
    import ml_dtypes

    f = np.float32
    bf = ml_dtypes.bfloat16
    f8 = ml_dtypes.float8_e4m3fn
    xq = np.ascontiguousarray(
        np.transpose(np.asarray(query, f), (0, 2, 1)).astype(f8)
    )
    xk = np.ascontiguousarray(
        np.transpose(np.asarray(key_, f), (0, 2, 1)).astype(f8)
    )
    xv = np.ascontiguousarray(
        np.transpose(np.asarray(value, f), (0, 2, 1)).astype(bf)
    )
    wo_t = np.ascontiguousarray(np.asarray(Wo, f).T.astype(bf))
    bo_r = np.ascontiguousarray(np.asarray(bo, f).reshape(1, D_MODEL).astype(bf))

    inv_freq = (
        1.0 / (ROPE_BASE ** (np.arange(0, HEAD_DIM, 2, dtype=f) / HEAD_DIM))
    ).astype(f)
    t = np.arange(S, dtype=f)
    freqs = np.einsum("i,j->ij", t, inv_freq).astype(f)  # [S, 32]
    emb = np.concatenate([freqs, freqs], axis=-1)  # [S, 64]
    cosT = np.cos(emb).astype(f).T  # [64, S]
    sinT = np.sin(emb).astype(f).T
    # sign folded: rows 0:32 get -sin (they receive -x2*sin), 32:64 get +sin
    sin_signed = np.concatenate([-sinT[0:32], sinT[32:64]], axis=0)
    cos_rep = np.ascontiguousarray(np.tile(cosT, (2, 1)).astype(bf))
    sin_rep = np.ascontiguousarray(np.tile(sin_signed, (2, 1)).astype(bf))

    # rotate-half permutation: perm[p, m] = 1 iff p = partner(m)
    perm = np.zeros((P, P), f)
    for m in range(P):
        base = (m // 64) * 64
        d = m - base
        partner = base + (d + 32 if d < 32 else d - 32)
        perm[partner, m] = 1.0
    perm = np.ascontiguousarray(perm.astype(bf))

    Wq, Wk, Wv = (np.asarray(w, f) for w in (Wq, Wk, Wv))
    bq, bk, bv = (np.asarray(v_, f) for v_ in (bq, bk, bv))

    in_maps = []
    for c in range(N_CORES):
        sl = slice(P * c, P * (c + 1))
        in_maps.append(
            {
                "xq": xq,
                "xk": xk,
                "xv": xv,
                "wq": np.ascontiguousarray(Wq[sl, :].T.astype(f8)),
                "wk": np.ascontiguousarray(Wk[sl, :].T.astype(f8)),
                "wv": np.ascontiguousarray(Wv[sl, :].T.astype(bf)),
                "bq": np.ascontiguousarray(bq[sl].reshape(P, 1)),
                "bk": np.ascontiguousarray(bk[sl].reshape(P, 1)),
                "bv": np.ascontiguousarray(bv[sl].reshape(1, P).astype(bf)),
                "wo": wo_t,
                "bo": bo_r,
                "cosr": cos_rep,
                "sinr": sin_rep,
                "perm": perm,
            }
        )
    return in_maps


def kernel(query, key_, value, Wq, bq, Wk, bk, Wv, bv, Wo, bo):
    global LAST_RESULTS
    from concourse.bass_utils import run_bass_kernel_spmd

    S = query.shape[1]
    in_maps = host_inputs(
        query, key_, value, Wq, bq, Wk, bk, Wv, bv, Wo, bo, S=S
    )
    nc = build_nc(S=S)
    res = run_bass_kernel_spmd(
        nc, in_maps, core_ids=list(range(N_CORES)), trace=TRACE
    )
    LAST_RESULTS = res
    TS = S // N_CORES
    out = np.empty((B, S, D_MODEL), np.float32)
    for c in range(N_CORES):
        out[:, TS * c : TS * (c + 1), :] = res.results[c]["y"]
    return out
